# revision 37
# baseline (speedup 1.0000x reference)
"""Trainium2 Bass kernel for nn_Attention_45303315038988.

  q = p @ Wh.T (+bh) ; k = r @ Wl.T + bl ; v = p @ Wg.T + bg     [N, D]
  scores = q @ k.T ; attn = softmax(scores, axis=0) ; out = p + attn @ v

Design (8 NeuronCores, sequence-parallel over the query/row axis):
  - Weight fusion: scores = q k^T = p (Wh^T Wl) r^T + (p Wh^T bl) 1^T
    + [per-key terms that softmax over the query axis cancels].
    Host precomputes M = Wh^T Wl (fp16) and pc = p_shard (Wh^T bl); the k
    projection and its AllGather disappear entirely — phase C contracts the
    raw r^T (full, feature-major fp16, streamed from local HBM) against
    q'^T = M^T p^T. pc is added into the scores PSUM by DVE against a
    one-time partition-replicated row tile (ones outer-product matmul);
    bg is added to v the same way.
  - Phase A runs db-outer two-pass projections (8 single-bank PSUM
    accumulators per pass) with operand loads round-robined over all three
    DMA rings so matmuls start as operand slices land; the v shard is
    AllGathered in fp16 as soon as it is computed.
  - scores^T = rT.T @ q'^T puts the softmax axis on the free dim: per-key
    max is a DVE reduction, E = exp(s + pc - m_local) goes fp16-resident in
    SBUF (16 MB) with the per-key sum accumulated for free by the ACT Exp
    pass (accum_out).
  - Softmax globalization: (max,sum) stats are AllGathered in two halves and
    combined locally; the correction exp(m_local - M)/S is folded into E by
    per-partition-scale ops alternating over the idle ACT/DVE streams at
    phase-E start. All cross-core sync points sit AFTER the phase-C loop in
    the engine FIFOs, so inter-core skew stalls nothing before phase E.
  - out = E^T.T @ V over 64 key blocks in two i-half passes (4 PSUM
    accumulators x 2 banks each); V streams fp16 with contiguous 2 KB rows;
    residual p tiles are prefetched at phase-E start; residual add in fp32.
  - Queue discipline (engine streams execute in program order, one DGE ring
    each for sync/scalar/gpsimd): collectives live alone on gpsimd; the rT
    stream splits sync (lo feature halves) / scalar ring (hi halves, whose
    triggers sit in the ACT stream where their pool-slot waits are already
    satisfied); V tiles split sync/gpsimd with the first 24 kept off the
    AllGather-blocked gpsimd queue; output stores fan out over all rings.
All matmul operands are fp16 with fp32 PSUM accumulation; softmax
statistics are fp32. Measured rel-to-absmax error 1.98e-3 (fp32 reference);
measured HW exec time ~616 us/run (8-core TRN2, NTFF-profiled).
"""
import numpy as np

P = 128
D = 1024
N = 8192
NCORES = 8
NL = N // NCORES
DB = D // P
JBL = NL // P
NG = N // P
IB = NL // P
FH = 512


def build_nc(k_iters: int = 1, no_cc: bool = False, phases: str = "full",
             opts: dict | None = None, spin_us: int = 0):
    opts = opts or {}
    import concourse.mybir as mybir
    import concourse.tile as tile
    from concourse import bacc

    f16 = mybir.dt.float16
    f32 = mybir.dt.float32
    AF = mybir.ActivationFunctionType
    AX = mybir.AxisListType
    ALU = mybir.AluOpType
    RG = [list(range(NCORES))]

    nc = bacc.Bacc("TRN2", target_bir_lowering=False, debug=False,
                   num_devices=1 if no_cc else NCORES)

    def collective(kind, op, ins, outs):
        if no_cc:
            src_ap, dst_ap = ins[0], outs[0]
            nc.sync.dma_start(out=dst_ap[0] if kind == "AllGather" else dst_ap[:],
                              in_=src_ap[:])
        else:
            nc.gpsimd.collective_compute(kind, op, replica_groups=RG,
                                         ins=[ins[0].opt()], outs=[outs[0].opt()])

    PM_h = nc.dram_tensor("PM16", [D, 2 * D], f16, kind="ExternalInput")
    rT_h = nc.dram_tensor("rTf", [D, N], f16, kind="ExternalInput")
    pres_h = nc.dram_tensor("pres", [NL, D], f32, kind="ExternalInput")
    WgT_h = nc.dram_tensor("WgT", [D, D], f16, kind="ExternalInput")
    pc_h = nc.dram_tensor("pc16", [1, NL], f16, kind="ExternalInput")
    bg_h = nc.dram_tensor("bg16", [1, D], f16, kind="ExternalInput")
    ones_h = nc.dram_tensor("ones16", [1, P], f16, kind="ExternalInput")
    out_h = nc.dram_tensor("out", [NL, D], f32, kind="ExternalOutput")

    with tile.TileContext(nc) as tc:
        with tc.tile_pool(name="dram", bufs=1, space="DRAM") as dpool:
            for it in range(k_iters):
                cc_v_in = dpool.tile([NL, D], f16, name=f"cc_v_in{it}")
                cc_v_out = dpool.tile([NCORES, NL, D], f16,
                                      addr_space="Shared", name=f"cc_v_out{it}")
                cc_st_in = [dpool.tile([P, NG], f32, name=f"cc_st_in{it}_{h}")
                            for h in range(2)]
                cc_st_out = [dpool.tile([NCORES, P, NG], f32, addr_space="Shared",
                                        name=f"cc_st_out{it}_{h}")
                             for h in range(2)]

                with tc.tile_pool(name="lp", bufs=1) as lp:
                    qT = lp.tile([P, DB, NL], f16)
                    stats = lp.tile([P, 2, NG], f32)   # [:,0,:]=-max, [:,1,:]=sum
                    f_sc = lp.tile([P, NG], f32)
                    pcrep = lp.tile([P, NL], f32)
                    bgrep = lp.tile([P, D], f16)
                    pc_sb = lp.tile([1, NL], f16)
                    bg_sb = lp.tile([1, D], f16)
                    ones_sb = lp.tile([1, P], f16)
                    nc.sync.dma_start(out=ones_sb, in_=ones_h.ap())
                    nc.sync.dma_start(out=pc_sb, in_=pc_h.ap())
                    nc.sync.dma_start(out=bg_sb, in_=bg_h.ap())

                    # ---------------- phase A: projections ----------------
                    # db-outer two-pass structure: each pass holds 8
                    # single-bank accumulators so the MMs for feature-block
                    # db can start as soon as its operand slices land.
                    with (
                        tc.tile_pool(name="pw", bufs=1) as pw,
                        tc.tile_pool(name="pst", bufs=1) as pst,
                    ):
                        PM_sb = pw.tile([P, DB, 2 * D], f16)
                        WgT_sb = pw.tile([P, DB, D], f16)
                        # host interleaves pT|M per row, so one DMA per db
                        # delivers the q'-projection pair in 4 KB packets
                        # (half the packet count of separate loads); pairs
                        # alternate the two early-starting rings, WgT rides
                        # the late-starting scalar ring for the later v pass
                        for db in range(DB):
                            eng = nc.sync if db % 2 == 0 else nc.gpsimd
                            eng.dma_start(
                                out=PM_sb[:, db, :],
                                in_=PM_h.ap()[db * P:(db + 1) * P, :])
                        for db in range(DB):
                            nc.scalar.dma_start(
                                out=WgT_sb[:, db, :],
                                in_=WgT_h.ap()[db * P:(db + 1) * P, :])

                        # pcrep = ones^T pc, bgrep = ones^T bg : one-time
                        with tc.tile_pool(name="psA0", bufs=2,
                                          space="PSUM") as psA0:
                            ps_t = psA0.tile([P, NL], f32)
                            for ih in range(2):
                                nc.tensor.matmul(
                                    ps_t[:, ih * FH:(ih + 1) * FH],
                                    lhsT=ones_sb[:, :],
                                    rhs=pc_sb[:, ih * FH:(ih + 1) * FH],
                                    start=True, stop=True)
                            nc.scalar.activation(out=pcrep, in_=ps_t,
                                                 func=AF.Copy)
                            ps_t = psA0.tile([P, NL], f32)
                            for ih in range(2):
                                nc.tensor.matmul(
                                    ps_t[:, ih * FH:(ih + 1) * FH],
                                    lhsT=ones_sb[:, :],
                                    rhs=bg_sb[:, ih * FH:(ih + 1) * FH],
                                    start=True, stop=True)
                            nc.scalar.activation(out=bgrep, in_=ps_t,
                                                 func=AF.Copy)

                        with tc.tile_pool(name="psA8", bufs=1,
                                          space="PSUM") as psA8:
                            # q'^T = M.T @ p^T first: the qT copies then land
                            # mid-phase-A (overlapped by the v matmuls) and
                            # phase C's handoff needs nothing but rt block 0
                            for ih in range(2):
                                cs = slice(ih * FH, (ih + 1) * FH)
                                ps = [psA8.tile([P, FH], f32, tag=f"a{j}",
                                                name=f"aq{j}")
                                      for j in range(DB)]
                                for db in range(DB):
                                    for dob in range(DB):
                                        nc.tensor.matmul(
                                            ps[dob],
                                            lhsT=PM_sb[:, db,
                                                       D + dob * P:
                                                       D + (dob + 1) * P],
                                            rhs=PM_sb[:, db, cs],
                                            start=(db == 0),
                                            stop=(db == DB - 1))
                                for dob in range(DB):
                                    nc.scalar.activation(
                                        out=qT[:, dob, cs], in_=ps[dob],
                                        func=AF.Copy)

                            # v shard = p^T.T @ Wg^T -> [j_l, dv]; +bg on DVE;
                            # its adds/stores trail off the PE critical path
                            st_tiles = [pst.tile([P, NL], f16, tag=f"st{jb}",
                                                  name=f"st{jb}")
                                        for jb in range(JBL)]
                            for dvh in range(2):
                                cs = slice(dvh * FH, (dvh + 1) * FH)
                                ps = [psA8.tile([P, FH], f32, tag=f"a{j}",
                                                name=f"av{j}")
                                      for j in range(JBL)]
                                for db in range(DB):
                                    for jb in range(JBL):
                                        nc.tensor.matmul(
                                            ps[jb],
                                            lhsT=PM_sb[:, db,
                                                       jb * P:(jb + 1) * P],
                                            rhs=WgT_sb[:, db, cs],
                                            start=(db == 0),
                                            stop=(db == DB - 1))
                                for jb in range(JBL):
                                    nc.vector.tensor_add(
                                        out=st_tiles[jb][:, cs], in0=ps[jb],
                                        in1=bgrep[:, cs])
                            for jb in range(JBL):
                                nc.gpsimd.dma_start(
                                    out=cc_v_in[jb * P:(jb + 1) * P, :],
                                    in_=st_tiles[jb])
                            collective("AllGather", ALU.bypass,
                                       [cc_v_in], [cc_v_out])

                    # -------- phase C: scores^T + local stats --------
                    # rT block loads are split across the sync and gpsimd
                    # queues (a single HWDGE ring tops out near the demand
                    # rate); the half-0 stats section is injected right after
                    # block 4's loads so its AllGather slots into the gpsimd
                    # queue between load groups and the E-rescale of the first
                    # half hides under phase C's second half.
                    ep_cm = tc.tile_pool(name="ep", bufs=1)
                    ep = ep_cm.__enter__()
                    E = ep.tile([P, NG, NL], f16)
                    NH = NG // 2
                    Mneg = lp.tile([P, NG], f32)
                    Ssum = lp.tile([P, NG], f32)
                    tmp = lp.tile([P, NG], f32)
                    diff = lp.tile([P, NG], f32)
                    alpha = lp.tile([P, NG], f32)
                    rec = lp.tile([P, NG], f32)
                    gath = [lp.tile([P, NCORES, 2, NH], f32, name=f"gath{h}")
                            for h in range(2)]

                    def stats_comm(h):
                        # stores + AllGather + combine; E-rescale is emitted
                        # separately (engine streams execute in program order,
                        # so rescale ops must be woven in by hand)
                        hs = slice(h * NH, (h + 1) * NH)
                        nc.gpsimd.dma_start(out=cc_st_in[h][:, 0:NH],
                                            in_=stats[:, 0, hs])
                        nc.gpsimd.dma_start(out=cc_st_in[h][:, NH:NG],
                                            in_=stats[:, 1, hs])
                        collective("AllGather", ALU.bypass,
                                   [cc_st_in[h]], [cc_st_out[h]])
                        nc.gpsimd.dma_start(
                            out=gath[h].rearrange("p c a b -> p (c a b)"),
                            in_=cc_st_out[h].rearrange("c p x -> p c x"))
                        g_h = gath[h]
                        nc.vector.tensor_copy(out=Mneg[:, hs], in_=g_h[:, 0, 0, :])
                        for c in range(1, NCORES):
                            nc.vector.tensor_tensor(out=Mneg[:, hs],
                                                    in0=Mneg[:, hs],
                                                    in1=g_h[:, c, 0, :],
                                                    op=ALU.min)
                        for c in range(NCORES):
                            nc.vector.tensor_sub(out=tmp[:, hs],
                                                 in0=Mneg[:, hs],
                                                 in1=g_h[:, c, 0, :])
                            nc.scalar.activation(out=tmp[:, hs],
                                                 in_=tmp[:, hs], func=AF.Exp)
                            nc.vector.tensor_mul(out=tmp[:, hs],
                                                 in0=tmp[:, hs],
                                                 in1=g_h[:, c, 1, :])
                            if c == 0:
                                nc.vector.tensor_copy(out=Ssum[:, hs],
                                                      in_=tmp[:, hs])
                            else:
                                nc.vector.tensor_add(out=Ssum[:, hs],
                                                     in0=Ssum[:, hs],
                                                     in1=tmp[:, hs])
                        # f = exp(Mneg - mneg_local) / Ssum, fold into E
                        nc.vector.tensor_sub(out=diff[:, hs], in0=Mneg[:, hs],
                                             in1=stats[:, 0, hs])
                        nc.scalar.activation(out=alpha[:, hs], in_=diff[:, hs],
                                             func=AF.Exp)
                        nc.vector.reciprocal(out=rec[:, hs], in_=Ssum[:, hs])
                        nc.vector.tensor_mul(out=f_sc[:, hs], in0=alpha[:, hs],
                                             in1=rec[:, hs])

                    def rescale(g, eng):
                        if eng is nc.scalar:
                            nc.scalar.activation(
                                out=E[:, g, :], in_=E[:, g, :], func=AF.Copy,
                                scale=f_sc[:, g:g + 1])
                        else:
                            eng.tensor_scalar_mul(out=E[:, g, :],
                                                  in0=E[:, g, :],
                                                  scalar1=f_sc[:, g:g + 1])

                    with (
                        tc.tile_pool(name="ktp", bufs=2) as ktp,
                        tc.tile_pool(name="psC", bufs=4, space="PSUM") as psC,
                    ):
                        rt_tiles = {}

                        def rt_load(blk):
                            # lo-halves ride sync; hi-halves ride the scalar
                            # HWDGE ring (the trigger sits in the ACT stream
                            # at a point where its pool-slot wait is already
                            # satisfied, so it never stalls the Exp ops) —
                            # keeps everything off gpsimd, which the stats
                            # AllGathers block for tens of us
                            rt_c = ktp.tile([P, DB, JBL, P], f16, tag="kt")
                            for db in range(DB):
                                # block 0 rides sync alone: its scalar-ring
                                # triggers would sit behind all of phase A's
                                # ACT work in that engine stream
                                eng = (nc.sync if (db < DB // 2 or blk == 0)
                                       else nc.scalar)
                                eng.dma_start(
                                    out=rt_c[:, db, :, :].rearrange(
                                        "p a b -> p (a b)"),
                                    in_=rT_h.ap()[db * P:(db + 1) * P,
                                                  blk * NL:(blk + 1) * NL])
                            rt_tiles[blk] = rt_c

                        def rt_groups(blk):
                            rt_c = rt_tiles.pop(blk)
                            for jlb in range(JBL):
                                g = blk * JBL + jlb
                                ps_t = psC.tile([P, NL], f32, tag="sc")
                                for db in range(DB):
                                    for ih in range(2):
                                        nc.tensor.matmul(
                                            ps_t[:, ih * FH:(ih + 1) * FH],
                                            lhsT=rt_c[:, db, jlb, :],
                                            rhs=qT[:, db, ih * FH:(ih + 1) * FH],
                                            start=(db == 0), stop=(db == DB - 1))
                                nc.vector.tensor_add(out=ps_t, in0=ps_t,
                                                     in1=pcrep)
                                nc.vector.tensor_reduce(
                                    out=stats[:, 0, g:g + 1], in_=ps_t,
                                    op=ALU.max, axis=AX.X, negate=True)
                                nc.scalar.activation(
                                    out=E[:, g, :], in_=ps_t, func=AF.Exp,
                                    bias=stats[:, 0, g:g + 1], scale=1.0,
                                    accum_out=stats[:, 1, g:g + 1])

                        # stats_comm comes AFTER the full loop: its gpsimd
                        # stores/AG still trigger at C-half (the queue is
                        # empty during C), but its combine ops no longer gate
                        # phase-C work in the ACT/DVE FIFOs — inter-core
                        # skew then stalls nothing before phase E
                        for blk in range(NCORES):
                            rt_load(blk)
                            rt_groups(blk)

                    # prefetch the first phase-E V tiles on gpsimd before the
                    # half-1 stats collective blocks that queue
                    vtp_cm = tc.tile_pool(name="vtp", bufs=8)
                    vtp = vtp_cm.__enter__()
                    vt_pre = []
                    for g in range(8):
                        c_idx, jlb = divmod(g, JBL)
                        vt = vtp.tile([P, D], f16, tag="vt", name=f"vtpre{g}")
                        nc.gpsimd.dma_start(
                            out=vt, in_=cc_v_out[c_idx, jlb * P:(jlb + 1) * P, :])
                        vt_pre.append(vt)
                    stats_comm(0)
                    stats_comm(1)
                    # rescales drain at phase-E start on the idle ACT/DVE
                    # streams, ascending so E[g] is ready before its matmuls
                    for g in range(NG):
                        rescale(g, nc.scalar if g % 2 == 0 else nc.vector)

                    # -------- phase E: out = E^T.T @ V + p (two i-half passes) --
                    with (
                        tc.tile_pool(name="prp", bufs=1) as prp,
                        tc.tile_pool(name="osp", bufs=2) as osp,
                        tc.tile_pool(name="psE", bufs=1, space="PSUM") as psE,
                    ):
                        for ihalf in range(2):
                            # prefetch this half's residual p tiles early so
                            # the output tail is just add+store
                            pr_tiles = []
                            for q_ in range(IB // 2):
                                ib = ihalf * (IB // 2) + q_
                                pr = prp.tile([P, D], f32, tag=f"pr{q_}")
                                nc.gpsimd.dma_start(
                                    out=pr, in_=pres_h.ap()[ib * P:(ib + 1) * P, :])
                                pr_tiles.append(pr)
                            po = [psE.tile([P, D], f32, tag=f"po{q_}",
                                           name=f"po{q_}")
                                  for q_ in range(IB // 2)]
                            for g in range(NG):
                                c_idx, jlb = divmod(g, JBL)
                                if ihalf == 0 and g < 8:
                                    vt = vt_pre[g]
                                else:
                                    vt = vtp.tile([P, D], f16, tag="vt")
                                    # keep early tiles off gpsimd (AG(st1)
                                    # still blocks that queue at phase-E start)
                                    eng = (nc.sync if (ihalf == 0 and g < 24)
                                           else (nc.sync if g % 2 else nc.gpsimd))
                                    eng.dma_start(
                                        out=vt,
                                        in_=cc_v_out[c_idx,
                                                     jlb * P:(jlb + 1) * P, :])
                                for q_ in range(IB // 2):
                                    ib = ihalf * (IB // 2) + q_
                                    for dvh in range(2):
                                        nc.tensor.matmul(
                                            po[q_][:, dvh * FH:(dvh + 1) * FH],
                                            lhsT=E[:, g, ib * P:(ib + 1) * P],
                                            rhs=vt[:, dvh * FH:(dvh + 1) * FH],
                                            start=(g == 0), stop=(g == NG - 1))
                            for q_ in range(IB // 2):
                                ib = ihalf * (IB // 2) + q_
                                # half-adds into one tile, then a single
                                # full-row store (4 KB DMA packets)
                                ot = osp.tile([P, D], f32, tag="ot")
                                for dvh in range(2):
                                    cs = slice(dvh * FH, (dvh + 1) * FH)
                                    nc.vector.tensor_add(out=ot[:, cs],
                                                         in0=po[q_][:, cs],
                                                         in1=pr_tiles[q_][:, cs])
                                eng = (nc.sync, nc.gpsimd, nc.scalar,
                                       nc.sync)[q_]
                                eng.dma_start(
                                    out=out_h.ap()[ib * P:(ib + 1) * P, :],
                                    in_=ot)
                    vtp_cm.__exit__(None, None, None)
                    ep_cm.__exit__(None, None, None)
            if spin_us:
                with tc.tile_critical():
                    for _ in range(spin_us):
                        nc.vector.nop(cycle_cnt=960)
    nc.compile()
    return nc


def prepare_in_maps(p, r, Wh, bh, Wl, bl, Wg, bg):
    f16 = np.float16
    f32 = np.float32
    p = np.asarray(p, dtype=f32)
    r = np.asarray(r, dtype=f32)
    Wh64 = np.asarray(Wh, dtype=np.float64)
    Wl64 = np.asarray(Wl, dtype=np.float64)
    M16 = np.ascontiguousarray(Wh64.T @ Wl64).astype(f16)
    c32 = (Wh64.T @ np.asarray(bl, dtype=np.float64)).astype(f32)
    WgT = np.ascontiguousarray(np.asarray(Wg).T).astype(f16)
    bg16 = np.asarray(bg).astype(f16).reshape(1, D)
    rTf = np.ascontiguousarray(r.T).astype(f16)
    in_maps = []
    for c in range(NCORES):
        sl = slice(c * NL, (c + 1) * NL)
        pc16 = (p[sl] @ c32).astype(f16).reshape(1, NL)
        PM16 = np.ascontiguousarray(
            np.concatenate([p[sl].T.astype(f16), M16], axis=1))
        in_maps.append({
            "PM16": PM16,
            "rTf": rTf,
            "pres": np.ascontiguousarray(p[sl]).astype(f32),
            "WgT": WgT, "pc16": pc16,
            "bg16": bg16, "ones16": np.ones((1, P), f16),
        })
    return in_maps


_NC_CACHE = {}


def kernel(p, r, Wh, bh, Wl, bl, Wg, bg):
    from concourse.bass_utils import run_bass_kernel_spmd

    p = np.asarray(p); r = np.asarray(r)
    in_maps = prepare_in_maps(p, r, np.asarray(Wh), np.asarray(bh),
                              np.asarray(Wl), np.asarray(bl),
                              np.asarray(Wg), np.asarray(bg))
    if 1 not in _NC_CACHE:
        _NC_CACHE[1] = build_nc(1)
    res = run_bass_kernel_spmd(_NC_CACHE[1], in_maps, list(range(NCORES)))
    out = np.concatenate([res.results[c]["out"] for c in range(NCORES)], axis=0)
    return out.astype(np.float32)


# revision 38
# speedup vs baseline: 1.0253x; 1.0253x over previous
"""Trainium2 Bass kernel for nn_Attention_45303315038988.

  q = p @ Wh.T (+bh) ; k = r @ Wl.T + bl ; v = p @ Wg.T + bg     [N, D]
  scores = q @ k.T ; attn = softmax(scores, axis=0) ; out = p + attn @ v

Design (8 NeuronCores, sequence-parallel over the query/row axis):
  - Weight fusion: scores = q k^T = p (Wh^T Wl) r^T + (p Wh^T bl) 1^T
    + [per-key terms that softmax over the query axis cancels].
    Host precomputes M = Wh^T Wl (fp16) and pc = p_shard (Wh^T bl); the k
    projection and its AllGather disappear entirely — phase C contracts the
    raw r^T (full, feature-major fp16, streamed from local HBM) against
    q'^T = M^T p^T. pc is added into the scores PSUM by DVE against a
    one-time partition-replicated row tile (ones outer-product matmul);
    bg is added to v the same way.
  - Phase A runs db-outer two-pass projections (8 single-bank PSUM
    accumulators per pass) with operand loads round-robined over all three
    DMA rings so matmuls start as operand slices land; the v shard is
    AllGathered in fp16 as soon as it is computed.
  - scores^T = rT.T @ q'^T puts the softmax axis on the free dim: per-key
    max is a DVE reduction, E = exp(s + pc - m_local) goes fp16-resident in
    SBUF (16 MB) with the per-key sum accumulated for free by the ACT Exp
    pass (accum_out).
  - Softmax globalization: (max,sum) stats are AllGathered in two halves and
    combined locally; the correction exp(m_local - M)/S is folded into E by
    per-partition-scale ops alternating over the idle ACT/DVE streams at
    phase-E start. All cross-core sync points sit AFTER the phase-C loop in
    the engine FIFOs, so inter-core skew stalls nothing before phase E.
  - out = E^T.T @ V over 64 key blocks in two i-half passes (4 PSUM
    accumulators x 2 banks each); V streams fp16 with contiguous 2 KB rows;
    residual p tiles are prefetched at phase-E start; residual add in fp32.
  - Queue discipline (engine streams execute in program order, one DGE ring
    each for sync/scalar/gpsimd): collectives live alone on gpsimd; the rT
    stream splits sync (lo feature halves) / scalar ring (hi halves, whose
    triggers sit in the ACT stream where their pool-slot waits are already
    satisfied); V tiles split sync/gpsimd with the first 24 kept off the
    AllGather-blocked gpsimd queue; output stores fan out over all rings.
All matmul operands are fp16 with fp32 PSUM accumulation; softmax
statistics are fp32. Measured rel-to-absmax error 1.98e-3 (fp32 reference);
measured HW exec time ~616 us/run (8-core TRN2, NTFF-profiled).
"""
import numpy as np

P = 128
D = 1024
N = 8192
NCORES = 8
NL = N // NCORES
DB = D // P
JBL = NL // P
NG = N // P
IB = NL // P
FH = 512


def build_nc(k_iters: int = 1, no_cc: bool = False, phases: str = "full",
             opts: dict | None = None, spin_us: int = 0):
    opts = opts or {}
    import concourse.mybir as mybir
    import concourse.tile as tile
    from concourse import bacc

    f16 = mybir.dt.float16
    f32 = mybir.dt.float32
    AF = mybir.ActivationFunctionType
    AX = mybir.AxisListType
    ALU = mybir.AluOpType
    RG = [list(range(NCORES))]

    nc = bacc.Bacc("TRN2", target_bir_lowering=False, debug=False,
                   num_devices=1 if no_cc else NCORES)

    def collective(kind, op, ins, outs):
        if no_cc:
            src_ap, dst_ap = ins[0], outs[0]
            nc.sync.dma_start(out=dst_ap[0] if kind == "AllGather" else dst_ap[:],
                              in_=src_ap[:])
        else:
            nc.gpsimd.collective_compute(kind, op, replica_groups=RG,
                                         ins=[ins[0].opt()], outs=[outs[0].opt()])

    PM_h = nc.dram_tensor("PM16", [D, 2 * D], f16, kind="ExternalInput")
    rT_h = nc.dram_tensor("rTf", [D, N], f16, kind="ExternalInput")
    pres_h = nc.dram_tensor("pres", [NL, D], f32, kind="ExternalInput")
    WgT_h = nc.dram_tensor("WgT", [D, D], f16, kind="ExternalInput")
    pc_h = nc.dram_tensor("pc16", [1, NL], f16, kind="ExternalInput")
    bg_h = nc.dram_tensor("bg16", [1, D], f16, kind="ExternalInput")
    ones_h = nc.dram_tensor("ones16", [1, P], f16, kind="ExternalInput")
    out_h = nc.dram_tensor("out", [NL, D], f32, kind="ExternalOutput")

    with tile.TileContext(nc) as tc:
        with tc.tile_pool(name="dram", bufs=1, space="DRAM") as dpool:
            for it in range(k_iters):
                cc_v_in = dpool.tile([NL, D], f16, name=f"cc_v_in{it}")
                cc_v_out = dpool.tile([NCORES, NL, D], f16,
                                      addr_space="Shared", name=f"cc_v_out{it}")
                cc_st_in = [dpool.tile([P, NG], f32, name=f"cc_st_in{it}_{h}")
                            for h in range(2)]
                cc_st_out = [dpool.tile([NCORES, P, NG], f32, addr_space="Shared",
                                        name=f"cc_st_out{it}_{h}")
                             for h in range(2)]

                with tc.tile_pool(name="lp", bufs=1) as lp:
                    qT = lp.tile([P, DB, NL], f16)
                    stats = lp.tile([P, 2, NG], f32)   # [:,0,:]=-max, [:,1,:]=sum
                    f_sc = lp.tile([P, NG], f32)
                    pcrep = lp.tile([P, NL], f32)
                    bgrep = lp.tile([P, D], f16)
                    pc_sb = lp.tile([1, NL], f16)
                    bg_sb = lp.tile([1, D], f16)
                    ones_sb = lp.tile([1, P], f16)
                    nc.sync.dma_start(out=ones_sb, in_=ones_h.ap())
                    nc.sync.dma_start(out=pc_sb, in_=pc_h.ap())
                    nc.sync.dma_start(out=bg_sb, in_=bg_h.ap())

                    # ---------------- phase A: projections ----------------
                    # db-outer two-pass structure: each pass holds 8
                    # single-bank accumulators so the MMs for feature-block
                    # db can start as soon as its operand slices land.
                    with (
                        tc.tile_pool(name="pw", bufs=1) as pw,
                        tc.tile_pool(name="pst", bufs=1) as pst,
                    ):
                        PM_sb = pw.tile([P, DB, 2 * D], f16)
                        WgT_sb = pw.tile([P, DB, D], f16)
                        # host interleaves pT|M per row, so one DMA per db
                        # delivers the q'-projection pair in 4 KB packets
                        # (half the packet count of separate loads); pairs
                        # alternate the two early-starting rings, WgT rides
                        # the late-starting scalar ring for the later v pass
                        for db in range(DB):
                            eng = nc.sync if db % 2 == 0 else nc.gpsimd
                            eng.dma_start(
                                out=PM_sb[:, db, :],
                                in_=PM_h.ap()[db * P:(db + 1) * P, :])
                        for db in range(DB):
                            nc.scalar.dma_start(
                                out=WgT_sb[:, db, :],
                                in_=WgT_h.ap()[db * P:(db + 1) * P, :])

                        # pcrep = ones^T pc, bgrep = ones^T bg : one-time
                        with tc.tile_pool(name="psA0", bufs=2,
                                          space="PSUM") as psA0:
                            ps_t = psA0.tile([P, NL], f32)
                            for ih in range(2):
                                nc.tensor.matmul(
                                    ps_t[:, ih * FH:(ih + 1) * FH],
                                    lhsT=ones_sb[:, :],
                                    rhs=pc_sb[:, ih * FH:(ih + 1) * FH],
                                    start=True, stop=True)
                            nc.scalar.activation(out=pcrep, in_=ps_t,
                                                 func=AF.Copy)
                            ps_t = psA0.tile([P, NL], f32)
                            for ih in range(2):
                                nc.tensor.matmul(
                                    ps_t[:, ih * FH:(ih + 1) * FH],
                                    lhsT=ones_sb[:, :],
                                    rhs=bg_sb[:, ih * FH:(ih + 1) * FH],
                                    start=True, stop=True)
                            nc.scalar.activation(out=bgrep, in_=ps_t,
                                                 func=AF.Copy)

                        with tc.tile_pool(name="psA8", bufs=1,
                                          space="PSUM") as psA8:
                            # q'^T = M.T @ p^T first: the qT copies then land
                            # mid-phase-A (overlapped by the v matmuls) and
                            # phase C's handoff needs nothing but rt block 0
                            for ih in range(2):
                                cs = slice(ih * FH, (ih + 1) * FH)
                                ps = [psA8.tile([P, FH], f32, tag=f"a{j}",
                                                name=f"aq{j}")
                                      for j in range(DB)]
                                for db in range(DB):
                                    for dob in range(DB):
                                        nc.tensor.matmul(
                                            ps[dob],
                                            lhsT=PM_sb[:, db,
                                                       D + dob * P:
                                                       D + (dob + 1) * P],
                                            rhs=PM_sb[:, db, cs],
                                            start=(db == 0),
                                            stop=(db == DB - 1))
                                for dob in range(DB):
                                    nc.scalar.activation(
                                        out=qT[:, dob, cs], in_=ps[dob],
                                        func=AF.Copy)

                            # v shard = p^T.T @ Wg^T -> [j_l, dv]; +bg on DVE;
                            # its adds/stores trail off the PE critical path
                            st_tiles = [pst.tile([P, NL], f16, tag=f"st{jb}",
                                                  name=f"st{jb}")
                                        for jb in range(JBL)]
                            for dvh in range(2):
                                cs = slice(dvh * FH, (dvh + 1) * FH)
                                ps = [psA8.tile([P, FH], f32, tag=f"a{j}",
                                                name=f"av{j}")
                                      for j in range(JBL)]
                                for db in range(DB):
                                    for jb in range(JBL):
                                        nc.tensor.matmul(
                                            ps[jb],
                                            lhsT=PM_sb[:, db,
                                                       jb * P:(jb + 1) * P],
                                            rhs=WgT_sb[:, db, cs],
                                            start=(db == 0),
                                            stop=(db == DB - 1))
                                        # last pass: emit each add right
                                        # after its accumulator stops so the
                                        # trailing adds don't gate the psC
                                        # bank handoff into phase C
                                        if dvh == 1 and db == DB - 1:
                                            nc.vector.tensor_add(
                                                out=st_tiles[jb][:, cs],
                                                in0=ps[jb], in1=bgrep[:, cs])
                                if dvh == 0:
                                    for jb in range(JBL):
                                        nc.vector.tensor_add(
                                            out=st_tiles[jb][:, cs],
                                            in0=ps[jb], in1=bgrep[:, cs])
                            for jb in range(JBL):
                                nc.gpsimd.dma_start(
                                    out=cc_v_in[jb * P:(jb + 1) * P, :],
                                    in_=st_tiles[jb])
                            collective("AllGather", ALU.bypass,
                                       [cc_v_in], [cc_v_out])

                    # -------- phase C: scores^T + local stats --------
                    # rT block loads are split across the sync and gpsimd
                    # queues (a single HWDGE ring tops out near the demand
                    # rate); the half-0 stats section is injected right after
                    # block 4's loads so its AllGather slots into the gpsimd
                    # queue between load groups and the E-rescale of the first
                    # half hides under phase C's second half.
                    ep_cm = tc.tile_pool(name="ep", bufs=1)
                    ep = ep_cm.__enter__()
                    E = ep.tile([P, NG, NL], f16)
                    NH = NG // 2
                    Mneg = lp.tile([P, NG], f32)
                    Ssum = lp.tile([P, NG], f32)
                    tmp = lp.tile([P, NG], f32)
                    diff = lp.tile([P, NG], f32)
                    alpha = lp.tile([P, NG], f32)
                    rec = lp.tile([P, NG], f32)
                    gath = [lp.tile([P, NCORES, 2, NH], f32, name=f"gath{h}")
                            for h in range(2)]

                    def stats_comm(h):
                        # stores + AllGather + combine; E-rescale is emitted
                        # separately (engine streams execute in program order,
                        # so rescale ops must be woven in by hand)
                        hs = slice(h * NH, (h + 1) * NH)
                        nc.gpsimd.dma_start(out=cc_st_in[h][:, 0:NH],
                                            in_=stats[:, 0, hs])
                        nc.gpsimd.dma_start(out=cc_st_in[h][:, NH:NG],
                                            in_=stats[:, 1, hs])
                        collective("AllGather", ALU.bypass,
                                   [cc_st_in[h]], [cc_st_out[h]])
                        nc.gpsimd.dma_start(
                            out=gath[h].rearrange("p c a b -> p (c a b)"),
                            in_=cc_st_out[h].rearrange("c p x -> p c x"))
                        g_h = gath[h]
                        nc.vector.tensor_copy(out=Mneg[:, hs], in_=g_h[:, 0, 0, :])
                        for c in range(1, NCORES):
                            nc.vector.tensor_tensor(out=Mneg[:, hs],
                                                    in0=Mneg[:, hs],
                                                    in1=g_h[:, c, 0, :],
                                                    op=ALU.min)
                        for c in range(NCORES):
                            nc.vector.tensor_sub(out=tmp[:, hs],
                                                 in0=Mneg[:, hs],
                                                 in1=g_h[:, c, 0, :])
                            nc.scalar.activation(out=tmp[:, hs],
                                                 in_=tmp[:, hs], func=AF.Exp)
                            nc.vector.tensor_mul(out=tmp[:, hs],
                                                 in0=tmp[:, hs],
                                                 in1=g_h[:, c, 1, :])
                            if c == 0:
                                nc.vector.tensor_copy(out=Ssum[:, hs],
                                                      in_=tmp[:, hs])
                            else:
                                nc.vector.tensor_add(out=Ssum[:, hs],
                                                     in0=Ssum[:, hs],
                                                     in1=tmp[:, hs])
                        # f = exp(Mneg - mneg_local) / Ssum, fold into E
                        nc.vector.tensor_sub(out=diff[:, hs], in0=Mneg[:, hs],
                                             in1=stats[:, 0, hs])
                        nc.scalar.activation(out=alpha[:, hs], in_=diff[:, hs],
                                             func=AF.Exp)
                        nc.vector.reciprocal(out=rec[:, hs], in_=Ssum[:, hs])
                        nc.vector.tensor_mul(out=f_sc[:, hs], in0=alpha[:, hs],
                                             in1=rec[:, hs])

                    def rescale(g, eng):
                        if eng is nc.scalar:
                            nc.scalar.activation(
                                out=E[:, g, :], in_=E[:, g, :], func=AF.Copy,
                                scale=f_sc[:, g:g + 1])
                        else:
                            eng.tensor_scalar_mul(out=E[:, g, :],
                                                  in0=E[:, g, :],
                                                  scalar1=f_sc[:, g:g + 1])

                    with (
                        tc.tile_pool(name="ktp", bufs=2) as ktp,
                        tc.tile_pool(name="psC", bufs=4, space="PSUM") as psC,
                    ):
                        rt_tiles = {}

                        def rt_load(blk):
                            # lo-halves ride sync; hi-halves ride the scalar
                            # HWDGE ring (the trigger sits in the ACT stream
                            # at a point where its pool-slot wait is already
                            # satisfied, so it never stalls the Exp ops) —
                            # keeps everything off gpsimd, which the stats
                            # AllGathers block for tens of us
                            rt_c = ktp.tile([P, DB, JBL, P], f16, tag="kt")
                            for db in range(DB):
                                # block 0 rides sync alone: its scalar-ring
                                # triggers would sit behind all of phase A's
                                # ACT work in that engine stream
                                eng = (nc.sync if (db < DB // 2 or blk == 0)
                                       else nc.scalar)
                                eng.dma_start(
                                    out=rt_c[:, db, :, :].rearrange(
                                        "p a b -> p (a b)"),
                                    in_=rT_h.ap()[db * P:(db + 1) * P,
                                                  blk * NL:(blk + 1) * NL])
                            rt_tiles[blk] = rt_c

                        def rt_groups(blk):
                            rt_c = rt_tiles.pop(blk)
                            for jlb in range(JBL):
                                g = blk * JBL + jlb
                                ps_t = psC.tile([P, NL], f32, tag="sc")
                                for db in range(DB):
                                    for ih in range(2):
                                        nc.tensor.matmul(
                                            ps_t[:, ih * FH:(ih + 1) * FH],
                                            lhsT=rt_c[:, db, jlb, :],
                                            rhs=qT[:, db, ih * FH:(ih + 1) * FH],
                                            start=(db == 0), stop=(db == DB - 1))
                                nc.vector.tensor_add(out=ps_t, in0=ps_t,
                                                     in1=pcrep)
                                nc.vector.tensor_reduce(
                                    out=stats[:, 0, g:g + 1], in_=ps_t,
                                    op=ALU.max, axis=AX.X, negate=True)
                                nc.scalar.activation(
                                    out=E[:, g, :], in_=ps_t, func=AF.Exp,
                                    bias=stats[:, 0, g:g + 1], scale=1.0,
                                    accum_out=stats[:, 1, g:g + 1])

                        # stats_comm comes AFTER the full loop: its gpsimd
                        # stores/AG still trigger at C-half (the queue is
                        # empty during C), but its combine ops no longer gate
                        # phase-C work in the ACT/DVE FIFOs — inter-core
                        # skew then stalls nothing before phase E
                        for blk in range(NCORES):
                            rt_load(blk)
                            rt_groups(blk)

                    # prefetch the first phase-E V tiles on gpsimd before the
                    # half-1 stats collective blocks that queue
                    vtp_cm = tc.tile_pool(name="vtp", bufs=8)
                    vtp = vtp_cm.__enter__()
                    vt_pre = []
                    for g in range(8):
                        c_idx, jlb = divmod(g, JBL)
                        vt = vtp.tile([P, D], f16, tag="vt", name=f"vtpre{g}")
                        nc.gpsimd.dma_start(
                            out=vt, in_=cc_v_out[c_idx, jlb * P:(jlb + 1) * P, :])
                        vt_pre.append(vt)
                    stats_comm(0)
                    stats_comm(1)
                    # rescales drain at phase-E start on the idle ACT/DVE
                    # streams, ascending so E[g] is ready before its matmuls
                    for g in range(NG):
                        rescale(g, nc.scalar if g % 2 == 0 else nc.vector)

                    # -------- phase E: out = E^T.T @ V + p (two i-half passes) --
                    with (
                        tc.tile_pool(name="prp", bufs=1) as prp,
                        tc.tile_pool(name="osp", bufs=2) as osp,
                        tc.tile_pool(name="psE", bufs=1, space="PSUM") as psE,
                    ):
                        for ihalf in range(2):
                            # prefetch this half's residual p tiles early so
                            # the output tail is just add+store
                            pr_tiles = []
                            for q_ in range(IB // 2):
                                ib = ihalf * (IB // 2) + q_
                                pr = prp.tile([P, D], f32, tag=f"pr{q_}")
                                nc.gpsimd.dma_start(
                                    out=pr, in_=pres_h.ap()[ib * P:(ib + 1) * P, :])
                                pr_tiles.append(pr)
                            po = [psE.tile([P, D], f32, tag=f"po{q_}",
                                           name=f"po{q_}")
                                  for q_ in range(IB // 2)]
                            for g in range(NG):
                                c_idx, jlb = divmod(g, JBL)
                                if ihalf == 0 and g < 8:
                                    vt = vt_pre[g]
                                else:
                                    vt = vtp.tile([P, D], f16, tag="vt")
                                    # keep early tiles off gpsimd (AG(st1)
                                    # blocks that queue at phase-E start) and
                                    # off scalar (rescales drain first); then
                                    # round-robin all three rings — two rings
                                    # alone sit right at the packet-rate edge
                                    eng = (nc.sync if (ihalf == 0 and g < 24)
                                           else (nc.sync, nc.gpsimd,
                                                 nc.scalar)[g % 3])
                                    eng.dma_start(
                                        out=vt,
                                        in_=cc_v_out[c_idx,
                                                     jlb * P:(jlb + 1) * P, :])
                                for q_ in range(IB // 2):
                                    ib = ihalf * (IB // 2) + q_
                                    for dvh in range(2):
                                        nc.tensor.matmul(
                                            po[q_][:, dvh * FH:(dvh + 1) * FH],
                                            lhsT=E[:, g, ib * P:(ib + 1) * P],
                                            rhs=vt[:, dvh * FH:(dvh + 1) * FH],
                                            start=(g == 0), stop=(g == NG - 1))
                            for q_ in range(IB // 2):
                                ib = ihalf * (IB // 2) + q_
                                # half-adds into one tile, then a single
                                # full-row store (4 KB DMA packets)
                                ot = osp.tile([P, D], f32, tag="ot")
                                for dvh in range(2):
                                    cs = slice(dvh * FH, (dvh + 1) * FH)
                                    nc.vector.tensor_add(out=ot[:, cs],
                                                         in0=po[q_][:, cs],
                                                         in1=pr_tiles[q_][:, cs])
                                eng = (nc.sync, nc.gpsimd, nc.scalar,
                                       nc.sync)[q_]
                                eng.dma_start(
                                    out=out_h.ap()[ib * P:(ib + 1) * P, :],
                                    in_=ot)
                    vtp_cm.__exit__(None, None, None)
                    ep_cm.__exit__(None, None, None)
            if spin_us:
                with tc.tile_critical():
                    for _ in range(spin_us):
                        nc.vector.nop(cycle_cnt=960)
    nc.compile()
    return nc


def prepare_in_maps(p, r, Wh, bh, Wl, bl, Wg, bg):
    f16 = np.float16
    f32 = np.float32
    p = np.asarray(p, dtype=f32)
    r = np.asarray(r, dtype=f32)
    Wh64 = np.asarray(Wh, dtype=np.float64)
    Wl64 = np.asarray(Wl, dtype=np.float64)
    M16 = np.ascontiguousarray(Wh64.T @ Wl64).astype(f16)
    c32 = (Wh64.T @ np.asarray(bl, dtype=np.float64)).astype(f32)
    WgT = np.ascontiguousarray(np.asarray(Wg).T).astype(f16)
    bg16 = np.asarray(bg).astype(f16).reshape(1, D)
    rTf = np.ascontiguousarray(r.T).astype(f16)
    in_maps = []
    for c in range(NCORES):
        sl = slice(c * NL, (c + 1) * NL)
        pc16 = (p[sl] @ c32).astype(f16).reshape(1, NL)
        PM16 = np.ascontiguousarray(
            np.concatenate([p[sl].T.astype(f16), M16], axis=1))
        in_maps.append({
            "PM16": PM16,
            "rTf": rTf,
            "pres": np.ascontiguousarray(p[sl]).astype(f32),
            "WgT": WgT, "pc16": pc16,
            "bg16": bg16, "ones16": np.ones((1, P), f16),
        })
    return in_maps


_NC_CACHE = {}


def kernel(p, r, Wh, bh, Wl, bl, Wg, bg):
    from concourse.bass_utils import run_bass_kernel_spmd

    p = np.asarray(p); r = np.asarray(r)
    in_maps = prepare_in_maps(p, r, np.asarray(Wh), np.asarray(bh),
                              np.asarray(Wl), np.asarray(bl),
                              np.asarray(Wg), np.asarray(bg))
    if 1 not in _NC_CACHE:
        _NC_CACHE[1] = build_nc(1)
    res = run_bass_kernel_spmd(_NC_CACHE[1], in_maps, list(range(NCORES)))
    out = np.concatenate([res.results[c]["out"] for c in range(NCORES)], axis=0)
    return out.astype(np.float32)
